# revision 27
# baseline (speedup 1.0000x reference)
"""Trainium2 Bass kernel for nn_CapsuleConv_4595615007178.

Math (reference): for input x[B,N,L,D], weights w[K,N,4,4,M]:
  nv[b,m,w,a,d] = (1/M) * sum_{n,k,x} x[b,n,w+k,a*4+x] * w[k,n,x,d,m]
  out = LayerNorm_{(a,d)}(nv) * gamma + beta      (eps=1e-5)

Device mapping (per core, data-parallel over batch, 2 batches/core),
all SBUF data in bf16 (fp32 PSUM accumulation):
  v[(m,d), (a,w)] = sum_k Wk[(n,x),(m,d)]^T @ X[(n,x), (a, w+k)]
  - one PSUM tile holds all 4 a-slices of a w-chunk; 3 matmuls (k) per chunk
  - ACT drains PSUM -> bf16 SBUF (one pass per chunk)
  - DVE: squares + pairwise-sum trees (4x/2x bf16 modes), layernorm stats
    reduced over d via PE matmul with an S matrix, per-m stats broadcast
    back to (m,d) partitions via PE with a B matrix, Pool drains those
  - rstd via ACT abs_reciprocal_sqrt table function
  - normalize: two full-span DVE STT passes; bf16 output, host upcasts
"""

import numpy as np

# problem dims (hardcoded per contest contract)
B, N, L, D = 16, 32, 2048, 16
M, OUT_D = 32, 16
K = 3
A, SQ = 4, 4
W = 2046
NCORES = 8
BPC = B // NCORES  # batches per core
EPS = 1e-5

CW = 512  # main-matmul chunk width along w (one PSUM bank per a-chain)
CH_MAIN = [(c0, min(CW, W - c0)) for c0 in range(0, W, CW)]
CH_STAT = [(0, 1024), (1024, W - 1024)]
CH_BC = CH_MAIN

_CACHE = {}


def _build(apply_gb, reps=1, chain=False, dump=False):
    import concourse.bacc as bacc
    import concourse.mybir as mybir
    from concourse import tile

    f32 = mybir.dt.float32
    bf16 = mybir.dt.bfloat16
    AL = mybir.AluOpType
    AF = mybir.ActivationFunctionType

    nc = bacc.Bacc("TRN2", target_bir_lowering=False, debug=False, num_devices=NCORES)
    xt = nc.dram_tensor("x", [BPC, 128, A, L], bf16, kind="ExternalInput")
    wt = nc.dram_tensor("w", [K, 128, 128], bf16, kind="ExternalInput")
    st = nc.dram_tensor("smat", [128, M], bf16, kind="ExternalInput")
    bt = nc.dram_tensor("bmat", [M, 128], bf16, kind="ExternalInput")
    if apply_gb:
        gbt = nc.dram_tensor("gb", [2, 128, A], f32, kind="ExternalInput")
    yt = nc.dram_tensor("y", [BPC, 128, A, W], bf16, kind="ExternalOutput")
    if dump:
        dvt = nc.dram_tensor("dv", [BPC, 128, A, W], bf16, kind="ExternalOutput")
        dvs = nc.dram_tensor("dvs", [BPC, 128, W], bf16, kind="ExternalOutput")
        dss = nc.dram_tensor("dss", [BPC, 128, W], bf16, kind="ExternalOutput")
        dmu = nc.dram_tensor("dmu", [BPC, 64, 1024], bf16, kind="ExternalOutput")
        dq = nc.dram_tensor("dq", [BPC, 64, 1024], bf16, kind="ExternalOutput")
        drs = nc.dram_tensor("drs", [BPC, 64, 1024], bf16, kind="ExternalOutput")
        dmr = nc.dram_tensor("dmr", [BPC, 128, 2, W], bf16, kind="ExternalOutput")

    with tile.TileContext(nc) as tc:
        with (
            tc.tile_pool(name="consts", bufs=1) as cpool,
            tc.tile_pool(name="xin", bufs=2) as xpool,
            tc.tile_pool(name="vbf", bufs=4) as vpool,
            tc.tile_pool(name="scr", bufs=2) as spool,  # sq / t scratch
            tc.tile_pool(name="outb", bufs=2) as opool,
            tc.tile_pool(name="stat", bufs=2) as stpool,
            tc.tile_pool(name="pmain", bufs=4, space="PSUM") as pmain,
            tc.tile_pool(name="pstat", bufs=2, space="PSUM") as pstat,
        ):
            w_sb = cpool.tile([128, K * 128], bf16)
            for k in range(K):
                nc.sync.dma_start(w_sb[:, k * 128 : (k + 1) * 128], wt[k])
            s_sb = cpool.tile([128, M], bf16)
            nc.sync.dma_start(s_sb[:], st[:])
            b_sb = cpool.tile([M, 128], bf16)
            nc.sync.dma_start(b_sb[:], bt[:])
            eps_sb = cpool.tile([64, 1], f32)
            nc.gpsimd.memset(eps_sb[:], EPS)
            if apply_gb:
                gb_sb = cpool.tile([128, 2 * A], f32)
                nc.sync.dma_start(gb_sb[:, 0:A], gbt[0])
                nc.sync.dma_start(gb_sb[:, A : 2 * A], gbt[1])

            pending = None
            for rep in range(reps):
                x_sb = {}
                v_bf = {}
                for b in range(BPC):
                    x_sb[b] = xpool.tile(
                        [128, A, L], bf16, tag="xin", name=f"x_{rep}_{b}"
                    )
                    nc.sync.dma_start(x_sb[b][:], xt[b])

                # phase 1: main matmuls + PSUM drains for both batches
                for b in range(BPC):
                    v_bf[b] = vpool.tile(
                        [128, A, W], bf16, tag="vbf", name=f"v_{rep}_{b}"
                    )
                    for ci, (c0, cwc) in enumerate(CH_MAIN):
                        pv = [
                            pmain.tile(
                                [128, CW], f32, tag="pv", bufs=4,
                                name=f"pv_{rep}_{b}_{ci}_{a}",
                            )
                            for a in range(A)
                        ]
                        for k in range(K):
                            for a in range(A):
                                nc.tensor.matmul(
                                    pv[a][:, :cwc],
                                    lhsT=w_sb[:, k * 128 : (k + 1) * 128],
                                    rhs=x_sb[b][:, a, k + c0 : k + c0 + cwc],
                                    start=(k == 0),
                                    stop=(k == K - 1),
                                )
                        for a in range(A):
                            nc.scalar.copy(
                                v_bf[b][:, a, c0 : c0 + cwc], pv[a][:, :cwc]
                            )

                # phase 2a1: squares for both batches (DVE, ahead of stats)
                sq_t = {}
                mr_t = {}
                ps_t = {}
                mu_t = {}
                rstd_t = {}
                for b in range(BPC):
                    sq = spool.tile([128, A, W], bf16, tag="scr", name=f"sq_{rep}_{b}")
                    sq_t[b] = sq
                    nc.vector.tensor_tensor(sq[:], v_bf[b][:], v_bf[b][:], op=AL.mult)

                # phase 2a2: stats matmuls for both batches
                for b in range(BPC):
                    v = v_bf[b]
                    sq = sq_t[b]
                    # per-m stats via PE: mu = S^T (sum_a v), q = S^T (sum_a v^2)
                    # with the a-sum folded into PSUM accumulation (4 rhs
                    # streams per piece). Two w-units packed at partition
                    # offsets 0/32: tile[(g*32+m), j] = stat[m, 1024*g + j]
                    ps_mu = pstat.tile(
                        [64, 1024], f32, tag="psmu", bufs=1, name=f"psmu_{rep}_{b}"
                    )
                    ps_q = pstat.tile(
                        [64, 1024], f32, tag="psq", bufs=1, name=f"psq_{rep}_{b}"
                    )
                    for g, (g0, gw) in enumerate(CH_STAT):
                        for h0 in range(0, gw, 512):
                            hw_ = min(512, gw - h0)
                            for a in range(A):
                                nc.tensor.matmul(
                                    ps_mu[32 * g : 32 * g + 32, h0 : h0 + hw_],
                                    lhsT=s_sb[:],
                                    rhs=v[:, a, g0 + h0 : g0 + h0 + hw_],
                                    start=(a == 0),
                                    stop=(a == A - 1),
                                )
                                nc.tensor.matmul(
                                    ps_q[32 * g : 32 * g + 32, h0 : h0 + hw_],
                                    lhsT=s_sb[:],
                                    rhs=sq[:, a, g0 + h0 : g0 + h0 + hw_],
                                    start=(a == 0),
                                    stop=(a == A - 1),
                                )
                    ps_t[b] = (ps_mu, ps_q)

                # phase 2a2: per-m smalls + broadcast for both batches
                for b in range(BPC):
                    ps_mu, ps_q = ps_t[b]
                    mu_bf = stpool.tile([64, 1024], bf16, tag="mu", bufs=4, name=f"mu_{rep}_{b}")
                    mu_t[b] = mu_bf
                    nc.scalar.copy(mu_bf[:], ps_mu[:])

                    # drain q to SBUF immediately (frees the PSUM bank for
                    # the other batch); variance math runs off-critical-path
                    q_f = stpool.tile([64, 1024], f32, tag="qf", bufs=2, name=f"qf_{rep}_{b}")
                    nc.scalar.copy(q_f[:], ps_q[:])
                    musq = stpool.tile([64, 1024], f32, tag="musq", bufs=1, name=f"musq_{rep}_{b}")
                    nc.vector.tensor_tensor(musq[:], mu_bf[:], mu_bf[:], op=AL.mult)
                    nc.vector.tensor_tensor(q_f[:], q_f[:], musq[:], op=AL.subtract)
                    rstd = stpool.tile([64, 1024], bf16, tag="rstd", bufs=4, name=f"rstd_{rep}_{b}")
                    rstd_t[b] = rstd
                    nc.scalar.activation(
                        rstd[:], q_f[:], AF.Abs_reciprocal_sqrt, bias=eps_sb[:]
                    )

                    mr_t[b] = (mu_bf, rstd)

                # phase 2b (deferred one rep): broadcast + normalize + output
                def _norm_phase(nrep, v_bf, mr_t):
                  if chain and nrep > 0:
                    ylook = cpool.tile(
                        [128, 1], bf16, tag="ylook", bufs=2, name=f"ylook_{nrep}"
                    )
                    nc.gpsimd.dma_start(ylook[:], yt[0][:, 0, 0:1])
                    for b in range(BPC):
                        rstd = mr_t[b][1]
                        nc.vector.scalar_tensor_tensor(
                            rstd[:], rstd[:], ylook[0:64, :], rstd[:],
                            op0=AL.mult, op1=AL.max,
                        )
                  mrs = {}
                  for b in range(BPC):
                    mu_bf, rstd = mr_t[b]
                    # broadcast mu, rstd to all (m,d) partitions via
                    # replicating SBUF->SBUF DMAs (partition stride 4);
                    # dispatched on the sync queue AFTER the next rep's x
                    # loads so they never delay input transfers.
                    mr = stpool.tile([128, 2, W], bf16, tag="mr", bufs=2, name=f"mr_{nrep}_{b}")
                    mrs[b] = mr
                    for g, (g0, gw) in enumerate(CH_STAT):
                        for d in range(SQ):
                            nc.sync.dma_start(
                                mr[d:128:4, 0, g0 : g0 + gw],
                                mu_bf[32 * g : 32 * g + 32, :gw],
                            )
                            nc.sync.dma_start(
                                mr[d:128:4, 1, g0 : g0 + gw],
                                rstd[32 * g : 32 * g + 32, :gw],
                            )
                  for b in range(BPC):
                    v = v_bf[b]
                    mr = mrs[b]
                    o_bf = opool.tile([128, A, W], bf16, tag="outb", name=f"o_{nrep}_{b}")
                    nc.vector.tensor_tensor(
                        o_bf[:], v[:], mr[:, 0:1, :].broadcast_to((128, A, W)),
                        op=AL.subtract,
                    )
                    rstdb = mr[:, 1:2, :].broadcast_to((128, A, W))
                    if apply_gb:
                        for a in range(A):
                            nc.vector.scalar_tensor_tensor(
                                o_bf[:, a, :],
                                o_bf[:, a, :],
                                gb_sb[:, a : a + 1],
                                mr[:, 1, :],
                                op0=AL.mult,
                                op1=AL.mult,
                            )
                            nc.vector.tensor_scalar(
                                o_bf[:, a, :],
                                o_bf[:, a, :],
                                gb_sb[:, A + a : A + a + 1],
                                None,
                                AL.add,
                            )
                    else:
                        nc.vector.tensor_tensor(o_bf[:], o_bf[:], rstdb, op=AL.mult)
                    nc.gpsimd.dma_start(yt[b], o_bf[:])

                if pending is not None:
                    _norm_phase(*pending)
                pending = (rep, dict(v_bf), dict(mr_t))

                if dump:
                  for b in range(BPC):
                    v = v_bf[b]
                    mu_bf = mu_t[b]
                    rstd = rstd_t[b]
                    if True:
                        nc.sync.dma_start(dvt[b], v[:])
                        nc.sync.dma_start(dmu[b], mu_bf[:])
                        nc.sync.dma_start(drs[b], rstd[:])

            if pending is not None:
                _norm_phase(*pending)

    nc.compile()
    return nc


def _get_nc(apply_gb):
    key = ("nc", apply_gb)
    if key not in _CACHE:
        _CACHE[key] = _build(apply_gb)
    return _CACHE[key]


def kernel(x, w, gamma, beta, num_iter=None, **_unused):
    import ml_dtypes
    from concourse.bass_utils import run_bass_kernel_spmd

    bf = ml_dtypes.bfloat16
    x = np.asarray(x, dtype=np.float32)
    w = np.asarray(w, dtype=np.float32)
    gamma = np.asarray(gamma, dtype=np.float32)
    beta = np.asarray(beta, dtype=np.float32)

    apply_gb = not (np.all(gamma == 1.0) and np.all(beta == 0.0))

    # host-side layout prep (pure permutation + dtype cast)
    xp = np.ascontiguousarray(
        x.reshape(B, N, L, A, SQ).transpose(0, 1, 4, 3, 2)
    ).reshape(B, 128, A, L).astype(bf)
    wp = (
        np.ascontiguousarray((w / float(M)).transpose(0, 1, 2, 4, 3))
        .reshape(K, 128, 128)
        .astype(bf)
    )
    smat = (np.kron(np.eye(M), np.ones((SQ, 1))) / float(OUT_D)).astype(bf)
    bmat = np.kron(np.eye(M), np.ones((1, SQ))).astype(bf)

    nc = _get_nc(apply_gb)

    in_maps = []
    for c in range(NCORES):
        m = {
            "x": xp[c * BPC : (c + 1) * BPC],
            "w": wp,
            "smat": smat,
            "bmat": bmat,
        }
        if apply_gb:
            gb = np.empty((2, 128, A), np.float32)
            for a in range(A):
                gb[0, :, a] = np.tile(gamma.reshape(A, SQ)[a], M)
                gb[1, :, a] = np.tile(beta.reshape(A, SQ)[a], M)
            m["gb"] = gb
        in_maps.append(m)

    res = run_bass_kernel_spmd(nc, in_maps, list(range(NCORES)))
    y = np.stack([np.asarray(res.results[c]["y"]) for c in range(NCORES)])
    # y: [8, BPC, 128, A, W] bf16 -> [B, M, W, 16] fp32
    y = y.astype(np.float32).reshape(B, M, SQ, A, W).transpose(0, 1, 4, 3, 2)
    return np.ascontiguousarray(y.reshape(B, M, W, OUT_D))


# revision 28
# speedup vs baseline: 1.0977x; 1.0977x over previous
"""Trainium2 Bass kernel for nn_CapsuleConv_4595615007178.

Math (reference): for input x[B,N,L,D], weights w[K,N,4,4,M]:
  nv[b,m,w,a,d] = (1/M) * sum_{n,k,x} x[b,n,w+k,a*4+x] * w[k,n,x,d,m]
  out = LayerNorm_{(a,d)}(nv) * gamma + beta      (eps=1e-5)

Device mapping (per core, data-parallel over batch, 2 batches/core),
all SBUF data in bf16 (fp32 PSUM accumulation):
  v[(m,d), (a,w)] = sum_k Wk[(n,x),(m,d)]^T @ X[(n,x), (a, w+k)]
  - one PSUM tile holds all 4 a-slices of a w-chunk; 3 matmuls (k) per chunk
  - ACT drains PSUM -> bf16 SBUF (one pass per chunk)
  - DVE: squares + pairwise-sum trees (4x/2x bf16 modes), layernorm stats
    reduced over d via PE matmul with an S matrix, per-m stats broadcast
    back to (m,d) partitions via PE with a B matrix, Pool drains those
  - rstd via ACT abs_reciprocal_sqrt table function
  - normalize: two full-span DVE STT passes; bf16 output, host upcasts
"""

import numpy as np

# problem dims (hardcoded per contest contract)
B, N, L, D = 16, 32, 2048, 16
M, OUT_D = 32, 16
K = 3
A, SQ = 4, 4
W = 2046
NCORES = 8
BPC = B // NCORES  # batches per core
EPS = 1e-5

CW = 512  # main-matmul chunk width along w (one PSUM bank per a-chain)
CH_MAIN = [(c0, min(CW, W - c0)) for c0 in range(0, W, CW)]
CH_STAT = [(0, 1024), (1024, W - 1024)]
CH_BC = CH_MAIN

_CACHE = {}


def _build(apply_gb, reps=1, chain=False, dump=False):
    import concourse.bacc as bacc
    import concourse.mybir as mybir
    from concourse import tile

    f32 = mybir.dt.float32
    bf16 = mybir.dt.bfloat16
    AL = mybir.AluOpType
    AF = mybir.ActivationFunctionType

    nc = bacc.Bacc("TRN2", target_bir_lowering=False, debug=False, num_devices=NCORES)
    xt = nc.dram_tensor("x", [BPC, 128, A, L], bf16, kind="ExternalInput")
    wt = nc.dram_tensor("w", [K, 128, 128], bf16, kind="ExternalInput")
    st = nc.dram_tensor("smat", [128, M], bf16, kind="ExternalInput")
    bt = nc.dram_tensor("bmat", [M, 128], bf16, kind="ExternalInput")
    if apply_gb:
        gbt = nc.dram_tensor("gb", [2, 128, A], f32, kind="ExternalInput")
    yt = nc.dram_tensor("y", [BPC, 128, A, W], bf16, kind="ExternalOutput")
    if dump:
        dvt = nc.dram_tensor("dv", [BPC, 128, A, W], bf16, kind="ExternalOutput")
        dvs = nc.dram_tensor("dvs", [BPC, 128, W], bf16, kind="ExternalOutput")
        dss = nc.dram_tensor("dss", [BPC, 128, W], bf16, kind="ExternalOutput")
        dmu = nc.dram_tensor("dmu", [BPC, 64, 1024], bf16, kind="ExternalOutput")
        dq = nc.dram_tensor("dq", [BPC, 64, 1024], bf16, kind="ExternalOutput")
        drs = nc.dram_tensor("drs", [BPC, 64, 1024], bf16, kind="ExternalOutput")
        dmr = nc.dram_tensor("dmr", [BPC, 128, 2, W], bf16, kind="ExternalOutput")

    with tile.TileContext(nc) as tc:
        with (
            tc.tile_pool(name="consts", bufs=1) as cpool,
            tc.tile_pool(name="xin", bufs=2) as xpool,
            tc.tile_pool(name="vbf", bufs=4) as vpool,
            tc.tile_pool(name="scr", bufs=2) as spool,  # sq / t scratch
            tc.tile_pool(name="outb", bufs=2) as opool,
            tc.tile_pool(name="stat", bufs=2) as stpool,
            tc.tile_pool(name="pmain", bufs=4, space="PSUM") as pmain,
            tc.tile_pool(name="pstat", bufs=2, space="PSUM") as pstat,
        ):
            w_sb = cpool.tile([128, K * 128], bf16)
            for k in range(K):
                nc.sync.dma_start(w_sb[:, k * 128 : (k + 1) * 128], wt[k])
            s_sb = cpool.tile([128, M], bf16)
            nc.sync.dma_start(s_sb[:], st[:])
            b_sb = cpool.tile([M, 128], bf16)
            nc.sync.dma_start(b_sb[:], bt[:])
            eps_sb = cpool.tile([64, 1], f32)
            nc.gpsimd.memset(eps_sb[:], EPS)
            if apply_gb:
                gb_sb = cpool.tile([128, 2 * A], f32)
                nc.sync.dma_start(gb_sb[:, 0:A], gbt[0])
                nc.sync.dma_start(gb_sb[:, A : 2 * A], gbt[1])

            pending = None
            for rep in range(reps):
                x_sb = {}
                v_bf = {}
                for b in range(BPC):
                    x_sb[b] = xpool.tile(
                        [128, A, L], bf16, tag="xin", name=f"x_{rep}_{b}"
                    )
                    nc.sync.dma_start(x_sb[b][:], xt[b])

                # phase 1: main matmuls + PSUM drains for both batches
                for b in range(BPC):
                    v_bf[b] = vpool.tile(
                        [128, A, W], bf16, tag="vbf", name=f"v_{rep}_{b}"
                    )
                    for ci, (c0, cwc) in enumerate(CH_MAIN):
                        pv = [
                            pmain.tile(
                                [128, CW], f32, tag="pv", bufs=4,
                                name=f"pv_{rep}_{b}_{ci}_{a}",
                            )
                            for a in range(A)
                        ]
                        for k in range(K):
                            for a in range(A):
                                nc.tensor.matmul(
                                    pv[a][:, :cwc],
                                    lhsT=w_sb[:, k * 128 : (k + 1) * 128],
                                    rhs=x_sb[b][:, a, k + c0 : k + c0 + cwc],
                                    start=(k == 0),
                                    stop=(k == K - 1),
                                )
                        for a in range(A):
                            nc.scalar.copy(
                                v_bf[b][:, a, c0 : c0 + cwc], pv[a][:, :cwc]
                            )

                # phase 2a1: squares for both batches (DVE, ahead of stats)
                sq_t = {}
                mr_t = {}
                ps_t = {}
                mu_t = {}
                rstd_t = {}
                for b in range(BPC):
                    sq = spool.tile([128, A, W], bf16, tag="scr", name=f"sq_{rep}_{b}")
                    sq_t[b] = sq
                    nc.vector.tensor_tensor(sq[:], v_bf[b][:], v_bf[b][:], op=AL.mult)

                # phase 2a2: stats matmuls for both batches
                for b in range(BPC):
                    v = v_bf[b]
                    sq = sq_t[b]
                    # per-m stats via PE: mu = S^T (sum_a v), q = S^T (sum_a v^2)
                    # with the a-sum folded into PSUM accumulation (4 rhs
                    # streams per piece). Two w-units packed at partition
                    # offsets 0/32: tile[(g*32+m), j] = stat[m, 1024*g + j]
                    ps_mu = pstat.tile(
                        [64, 1024], f32, tag="psmu", bufs=1, name=f"psmu_{rep}_{b}"
                    )
                    ps_q = pstat.tile(
                        [64, 1024], f32, tag="psq", bufs=1, name=f"psq_{rep}_{b}"
                    )
                    for g, (g0, gw) in enumerate(CH_STAT):
                        for h0 in range(0, gw, 512):
                            hw_ = min(512, gw - h0)
                            for a in range(A):
                                nc.tensor.matmul(
                                    ps_mu[32 * g : 32 * g + 32, h0 : h0 + hw_],
                                    lhsT=s_sb[:],
                                    rhs=v[:, a, g0 + h0 : g0 + h0 + hw_],
                                    start=(a == 0),
                                    stop=(a == A - 1),
                                )
                                nc.tensor.matmul(
                                    ps_q[32 * g : 32 * g + 32, h0 : h0 + hw_],
                                    lhsT=s_sb[:],
                                    rhs=sq[:, a, g0 + h0 : g0 + h0 + hw_],
                                    start=(a == 0),
                                    stop=(a == A - 1),
                                )
                    ps_t[b] = (ps_mu, ps_q)

                # phase 2a2: per-m smalls + broadcast for both batches
                for b in range(BPC):
                    ps_mu, ps_q = ps_t[b]
                    mu_bf = stpool.tile([64, 1024], bf16, tag="mu", bufs=4, name=f"mu_{rep}_{b}")
                    mu_t[b] = mu_bf
                    nc.scalar.copy(mu_bf[:], ps_mu[:])

                    # variance in fp32 straight from PSUM (bf16 only for v/rstd)
                    musq = stpool.tile([64, 1024], f32, tag="musq", bufs=1, name=f"musq_{rep}_{b}")
                    nc.vector.tensor_tensor(musq[:], mu_bf[:], mu_bf[:], op=AL.mult)
                    vare = stpool.tile([64, 1024], f32, tag="vare", bufs=1, name=f"vare_{rep}_{b}")
                    nc.vector.tensor_tensor(vare[:], ps_q[:], musq[:], op=AL.subtract)
                    rstd = stpool.tile([64, 1024], bf16, tag="rstd", bufs=4, name=f"rstd_{rep}_{b}")
                    rstd_t[b] = rstd
                    nc.scalar.activation(
                        rstd[:], vare[:], AF.Abs_reciprocal_sqrt, bias=eps_sb[:]
                    )

                    mr_t[b] = (mu_bf, rstd)

                # phase 2b (deferred one rep): broadcast + normalize + output
                def _norm_phase(nrep, v_bf, mr_t):
                  if chain and nrep > 0:
                    ylook = cpool.tile(
                        [128, 1], bf16, tag="ylook", bufs=2, name=f"ylook_{nrep}"
                    )
                    nc.gpsimd.dma_start(ylook[:], yt[0][:, 0, 0:1])
                    for b in range(BPC):
                        rstd = mr_t[b][1]
                        nc.vector.scalar_tensor_tensor(
                            rstd[:], rstd[:], ylook[0:64, :], rstd[:],
                            op0=AL.mult, op1=AL.max,
                        )
                  mrs = {}
                  for b in range(BPC):
                    mu_bf, rstd = mr_t[b]
                    # broadcast mu, rstd to all (m,d) partitions via
                    # replicating SBUF->SBUF DMAs (partition stride 4);
                    # dispatched on the sync queue AFTER the next rep's x
                    # loads so they never delay input transfers.
                    mr = stpool.tile([128, 2, W], bf16, tag="mr", bufs=2, name=f"mr_{nrep}_{b}")
                    mrs[b] = mr
                    for g, (g0, gw) in enumerate(CH_STAT):
                        for d in range(SQ):
                            nc.sync.dma_start(
                                mr[d:128:4, 0, g0 : g0 + gw],
                                mu_bf[32 * g : 32 * g + 32, :gw],
                            )
                            nc.sync.dma_start(
                                mr[d:128:4, 1, g0 : g0 + gw],
                                rstd[32 * g : 32 * g + 32, :gw],
                            )
                  for b in range(BPC):
                    v = v_bf[b]
                    mr = mrs[b]
                    o_bf = opool.tile([128, A, W], bf16, tag="outb", name=f"o_{nrep}_{b}")
                    nc.vector.tensor_tensor(
                        o_bf[:], v[:], mr[:, 0:1, :].broadcast_to((128, A, W)),
                        op=AL.subtract,
                    )
                    rstdb = mr[:, 1:2, :].broadcast_to((128, A, W))
                    if apply_gb:
                        for a in range(A):
                            nc.vector.scalar_tensor_tensor(
                                o_bf[:, a, :],
                                o_bf[:, a, :],
                                gb_sb[:, a : a + 1],
                                mr[:, 1, :],
                                op0=AL.mult,
                                op1=AL.mult,
                            )
                            nc.vector.tensor_scalar(
                                o_bf[:, a, :],
                                o_bf[:, a, :],
                                gb_sb[:, A + a : A + a + 1],
                                None,
                                AL.add,
                            )
                    else:
                        nc.vector.tensor_tensor(o_bf[:], o_bf[:], rstdb, op=AL.mult)
                    nc.gpsimd.dma_start(yt[b], o_bf[:])

                if pending is not None:
                    _norm_phase(*pending)
                pending = (rep, dict(v_bf), dict(mr_t))

                if dump:
                  for b in range(BPC):
                    v = v_bf[b]
                    mu_bf = mu_t[b]
                    rstd = rstd_t[b]
                    if True:
                        nc.sync.dma_start(dvt[b], v[:])
                        nc.sync.dma_start(dmu[b], mu_bf[:])
                        nc.sync.dma_start(drs[b], rstd[:])

            if pending is not None:
                _norm_phase(*pending)

    nc.compile()
    return nc


def _get_nc(apply_gb):
    key = ("nc", apply_gb)
    if key not in _CACHE:
        _CACHE[key] = _build(apply_gb)
    return _CACHE[key]


def kernel(x, w, gamma, beta, num_iter=None, **_unused):
    import ml_dtypes
    from concourse.bass_utils import run_bass_kernel_spmd

    bf = ml_dtypes.bfloat16
    x = np.asarray(x, dtype=np.float32)
    w = np.asarray(w, dtype=np.float32)
    gamma = np.asarray(gamma, dtype=np.float32)
    beta = np.asarray(beta, dtype=np.float32)

    apply_gb = not (np.all(gamma == 1.0) and np.all(beta == 0.0))

    # host-side layout prep (pure permutation + dtype cast)
    xp = np.ascontiguousarray(
        x.reshape(B, N, L, A, SQ).transpose(0, 1, 4, 3, 2)
    ).reshape(B, 128, A, L).astype(bf)
    wp = (
        np.ascontiguousarray((w / float(M)).transpose(0, 1, 2, 4, 3))
        .reshape(K, 128, 128)
        .astype(bf)
    )
    smat = (np.kron(np.eye(M), np.ones((SQ, 1))) / float(OUT_D)).astype(bf)
    bmat = np.kron(np.eye(M), np.ones((1, SQ))).astype(bf)

    nc = _get_nc(apply_gb)

    in_maps = []
    for c in range(NCORES):
        m = {
            "x": xp[c * BPC : (c + 1) * BPC],
            "w": wp,
            "smat": smat,
            "bmat": bmat,
        }
        if apply_gb:
            gb = np.empty((2, 128, A), np.float32)
            for a in range(A):
                gb[0, :, a] = np.tile(gamma.reshape(A, SQ)[a], M)
                gb[1, :, a] = np.tile(beta.reshape(A, SQ)[a], M)
            m["gb"] = gb
        in_maps.append(m)

    res = run_bass_kernel_spmd(nc, in_maps, list(range(NCORES)))
    y = np.stack([np.asarray(res.results[c]["y"]) for c in range(NCORES)])
    # y: [8, BPC, 128, A, W] bf16 -> [B, M, W, 16] fp32
    y = y.astype(np.float32).reshape(B, M, SQ, A, W).transpose(0, 1, 4, 3, 2)
    return np.ascontiguousarray(y.reshape(B, M, W, OUT_D))
